# revision 12
# baseline (speedup 1.0000x reference)
"""A3C loss kernel for Trainium2 (8 NeuronCores, data-parallel over batch).

The reference is a reverse scan over T=128 timesteps per trajectory:
    R_t   = sum_{s>=t} g^(s-t) r_s + g^(T-t) R0
    gae_t = R_t - v_t  (lambda=1 GAE telescopes to the advantage)
    critic = 0.5 * sum_t (R_t - v_t)^2
    actor  = -sum_t lp_t * gae_t - beta * sum_{t,a} ent
The suffix scan is a matmul with a [T,T] discount matrix, so the whole loss
is DMA + reductions + one small matmul per 128-row block.

The kernel is HBM-bound (~75.5 MB of input per core ≈ 211 us at 358 GB/s),
so the layout is organized around keeping the DMA rings saturated:
  - log_probs/entropies stream in groups of 4 row-blocks (2 MiB per DMA,
    16 KiB contiguous per partition), triple buffered, split across the two
    HWDGE rings (sync carries lp, scalar carries ent).
  - values/rewards are prefetched in 1 MiB chunks interleaved into the same
    rings ahead of the groups that consume them.
  - all elementwise/reduce work is batched per group (one op per group
    instead of one per block) and spread across vector/scalar/gpsimd so no
    engine comes near the DMA floor.

Layout: each core owns BC=8192 rows; block k (0..63), partition p holds local
row p*64+k, which makes every grouped load contiguous per partition and the
final [BC,2] output a single contiguous DMA from a [128, 128] staging tile.
"""

import numpy as np
from contextlib import ExitStack

import concourse.bacc as bacc
import concourse.bass as bass
import concourse.tile as tile
from concourse import mybir
from concourse.bass_utils import run_bass_kernel_spmd

GAMMA = 0.99
BETA = 0.01
B, T, A = 65536, 128, 8
N_CORES = 8
BC = B // N_CORES

G = 4  # row-blocks per streamed load group

F32 = mybir.dt.float32
ALU = mybir.AluOpType
ACTF = mybir.ActivationFunctionType
AXIS_X = mybir.AxisListType.X


def _discount_matrix() -> np.ndarray:
    # L[s, t] = gamma^(s-t) for s >= t else 0
    s = np.arange(T, dtype=np.float64)[:, None]
    t = np.arange(T, dtype=np.float64)[None, :]
    m = np.where(s >= t, GAMMA ** np.maximum(s - t, 0.0), 0.0)
    return m.astype(np.float32)


def build_nc(bc: int = BC):
    kb = bc // 128          # row-blocks
    ng = kb // G            # streamed groups
    nch = max(1, kb // 16)  # v/r chunks (16 blocks = 1 MiB of rewards each)
    jb = kb // nch          # blocks per chunk
    gpc = ng // nch         # groups per chunk
    assert bc % 128 == 0 and kb % G == 0 and ng % nch == 0 and jb % G == 0

    nc = bacc.Bacc("TRN2", target_bir_lowering=False, debug=False)

    v_d = nc.dram_tensor("values", [bc, T], F32, kind="ExternalInput")
    lv_d = nc.dram_tensor("last_value", [bc], F32, kind="ExternalInput")
    r_d = nc.dram_tensor("rewards", [bc, T], F32, kind="ExternalInput")
    lp_d = nc.dram_tensor("log_probs", [bc, T, A], F32, kind="ExternalInput")
    en_d = nc.dram_tensor("entropies", [bc, T, A], F32, kind="ExternalInput")
    tm_d = nc.dram_tensor("terminal_mask", [bc], mybir.dt.uint8, kind="ExternalInput")
    out_d = nc.dram_tensor("out", [bc, 2], F32, kind="ExternalOutput")

    lgam_d = nc.inline_tensor(_discount_matrix(), "lgam")
    iden_d = nc.inline_tensor(np.eye(128, dtype=np.float32), "iden")

    # local row index = p*kb + g*G + b -> grouped loads are contiguous per
    # partition (G and jb consecutive rows respectively)
    lp_view = lp_d.rearrange("(p g b) t a -> g p (b t) a", g=ng, b=G)
    en_view = en_d.rearrange("(p g b) t a -> g p b (t a)", g=ng, b=G)
    v_view = v_d.rearrange("(p c j) t -> c p (j t)", c=nch, j=jb)
    r_view = r_d.rearrange("(p c j) t -> c p j t", c=nch, j=jb)
    lv_view = lv_d.rearrange("(p k) -> p k", k=kb)
    tm_view = tm_d.rearrange("(p k) -> p k", k=kb)
    out_view = out_d.rearrange("(p k) j -> p (k j)", k=kb)

    with tile.TileContext(nc) as tc, ExitStack() as ctx:
        singles = ctx.enter_context(tc.tile_pool(name="singles", bufs=1))
        lp_pool = ctx.enter_context(tc.tile_pool(name="lp", bufs=3))
        en_pool = ctx.enter_context(tc.tile_pool(name="en", bufs=4))
        vr_pool = ctx.enter_context(tc.tile_pool(name="vr", bufs=nch))
        work = ctx.enter_context(tc.tile_pool(name="work", bufs=2))
        small = ctx.enter_context(tc.tile_pool(name="small", bufs=4))
        psum_t = ctx.enter_context(tc.tile_pool(name="psum_t", bufs=2, space="PSUM"))
        psum_r = ctx.enter_context(tc.tile_pool(name="psum_r", bufs=2, space="PSUM"))

        # singles go through SWDGE (gpsimd) so the two HWDGE rings are free
        # for the streamed loads from instruction zero
        lgam_s = singles.tile([128, 128], F32)
        nc.gpsimd.dma_start(out=lgam_s, in_=lgam_d[:])
        iden_s = singles.tile([128, 128], F32)
        nc.gpsimd.dma_start(out=iden_s, in_=iden_d[:])
        lv_s = singles.tile([128, kb], F32)
        nc.gpsimd.dma_start(out=lv_s, in_=lv_view)
        tm_s = singles.tile([128, kb], mybir.dt.uint8)
        nc.gpsimd.dma_start(out=tm_s, in_=tm_view)

        # gr0 = gamma * last_value * (1 - mask)
        tmf = singles.tile([128, kb], F32)
        nc.gpsimd.tensor_copy(out=tmf, in_=tm_s)
        lvm = singles.tile([128, kb], F32)
        nc.gpsimd.tensor_mul(lvm, lv_s, tmf)
        gr0 = singles.tile([128, kb], F32)
        nc.gpsimd.tensor_sub(gr0, lv_s, lvm)
        nc.gpsimd.tensor_scalar_mul(gr0, gr0, GAMMA)

        stage = singles.tile([128, 2 * kb], F32)
        stage_kj = stage.rearrange("p (k j) -> p k j", j=2)

        v_t: list = []
        r_t: list = []

        for g in range(ng):
            if g % gpc == 0:
                # prefetch the next v/r chunk into each ring ahead of the
                # groups that consume it; fold gamma*R0 into the last reward
                # column once per chunk (one strided op instead of per-block)
                c = g // gpc
                rt = vr_pool.tile([128, jb, T], F32)
                nc.sync.dma_start(out=rt, in_=r_view[c])
                vt = vr_pool.tile([128, jb * T], F32)
                nc.scalar.dma_start(out=vt, in_=v_view[c])
                nc.gpsimd.tensor_tensor(
                    out=rt[:, :, T - 1],
                    in0=rt[:, :, T - 1],
                    in1=gr0[:, c * jb : (c + 1) * jb],
                    op=ALU.add,
                )
                r_t.append(rt)
                v_t.append(vt)

            c = g // gpc
            j0 = (g % gpc) * G
            k0 = g * G

            lp_t = lp_pool.tile([128, G * T, A], F32)
            nc.sync.dma_start(out=lp_t, in_=lp_view[g])
            en_t = en_pool.tile([128, G, T * A], F32)
            nc.scalar.dma_start(out=en_t, in_=en_view[g])

            # lp[b, t] = sum_a log_probs
            lps = work.tile([128, G * T], F32)
            nc.vector.reduce_sum(out=lps, in_=lp_t, axis=AXIS_X)

            # ents[b] = sum_{t,a} entropies; beta is folded into the final
            # combine. Kept on vector: the scalar engine must stay off the
            # cross-engine dependency loop since it issues the ent/v DMAs —
            # any compute it does that waits on vector stalls the ring.
            ents = small.tile([128, G], F32)
            nc.vector.reduce_sum(out=ents, in_=en_t, axis=AXIS_X)

            # time-major rewards for the scan matmuls
            rt_ps = psum_t.tile([128, G * T], F32)
            for b in range(G):
                nc.tensor.transpose(
                    rt_ps[:, b * T : (b + 1) * T], r_t[c][:, j0 + b, :], iden_s
                )
            rt_sb = work.tile([128, G * T], F32)
            nc.scalar.activation(out=rt_sb, in_=rt_ps, func=ACTF.Copy, bias=0.0, scale=1.0)

            # R[b, t] = sum_s r'T[s, b] * Lgam[s, t]
            R_ps = psum_r.tile([128, G * T], F32)
            for b in range(G):
                nc.tensor.matmul(
                    R_ps[:, b * T : (b + 1) * T],
                    lhsT=rt_sb[:, b * T : (b + 1) * T],
                    rhs=lgam_s,
                    start=True,
                    stop=True,
                )

            adv = work.tile([128, G * T], F32)
            nc.vector.tensor_sub(adv, R_ps, v_t[c][:, j0 * T : (j0 + G) * T])

            # critic = 0.5 * sum_t adv^2 (square on vector, not scalar — see
            # the ents comment)
            sq = work.tile([128, G * T], F32)
            nc.vector.scalar_tensor_tensor(
                out=sq, in0=adv, scalar=0.5, in1=adv,
                op0=ALU.mult, op1=ALU.mult,
            )
            nc.vector.reduce_sum(
                out=stage_kj[:, k0 : k0 + G, 1],
                in_=sq.rearrange("p (b t) -> p b t", t=T),
                axis=AXIS_X,
            )

            # actor = -sum_t lp*adv - beta*sum ent
            prod = work.tile([128, G * T], F32)
            nc.vector.scalar_tensor_tensor(
                out=prod, in0=adv, scalar=-1.0, in1=lps,
                op0=ALU.mult, op1=ALU.mult,
            )
            acc = small.tile([128, G], F32)
            nc.vector.reduce_sum(
                out=acc, in_=prod.rearrange("p (b t) -> p b t", t=T), axis=AXIS_X
            )
            nc.vector.scalar_tensor_tensor(
                out=stage_kj[:, k0 : k0 + G, 0], in0=ents, scalar=-BETA, in1=acc,
                op0=ALU.mult, op1=ALU.add,
            )

        # single store at the end on the sync ring (drained of loads by then)
        nc.sync.dma_start(out=out_view, in_=stage)

    nc.compile()
    return nc


_NC = None


def _get_nc():
    global _NC
    if _NC is None:
        _NC = build_nc(BC)
    return _NC


def _make_in_maps(inputs: dict) -> list[dict]:
    v = np.ascontiguousarray(np.asarray(inputs["values"], dtype=np.float32))
    lv = np.ascontiguousarray(np.asarray(inputs["last_value"], dtype=np.float32))
    r = np.ascontiguousarray(np.asarray(inputs["rewards"], dtype=np.float32))
    lp = np.ascontiguousarray(np.asarray(inputs["log_probs"], dtype=np.float32))
    en = np.ascontiguousarray(np.asarray(inputs["entropies"], dtype=np.float32))
    tm = np.ascontiguousarray(np.asarray(inputs["terminal_mask"]).astype(np.uint8))
    maps = []
    for c in range(N_CORES):
        sl = slice(c * BC, (c + 1) * BC)
        maps.append(
            {
                "values": v[sl],
                "last_value": lv[sl],
                "rewards": r[sl],
                "log_probs": lp[sl],
                "entropies": en[sl],
                "terminal_mask": tm[sl],
            }
        )
    return maps


def _run(inputs: dict, trace: bool = False):
    nc = _get_nc()
    res = run_bass_kernel_spmd(
        nc,
        _make_in_maps(inputs),
        core_ids=list(range(N_CORES)),
        trace=trace,
    )
    out = np.concatenate([res.results[c]["out"] for c in range(N_CORES)], axis=0)
    return out, res


def kernel(**inputs) -> np.ndarray:
    out, _ = _run(inputs, trace=False)
    return out


# revision 15
# speedup vs baseline: 1.0459x; 1.0459x over previous
"""A3C loss kernel for Trainium2 (8 NeuronCores, data-parallel over batch).

The reference is a reverse scan over T=128 timesteps per trajectory:
    R_t   = sum_{s>=t} g^(s-t) r_s + g^(T-t) R0
    gae_t = R_t - v_t  (lambda=1 GAE telescopes to the advantage)
    critic = 0.5 * sum_t (R_t - v_t)^2
    actor  = -sum_t lp_t * gae_t - beta * sum_{t,a} ent
The suffix scan is a matmul with a [T,T] discount matrix, so the whole loss
is DMA + reductions + one small matmul per 128-row block.

The kernel is HBM-bound (~75.5 MB of input per core ≈ 211 us at 358 GB/s),
so the layout is organized around keeping the DMA rings saturated:
  - log_probs/entropies stream in groups of 4 row-blocks (2 MiB per DMA,
    16 KiB contiguous per partition), triple buffered, split across the two
    HWDGE rings (sync carries lp, scalar carries ent).
  - values/rewards are prefetched in 1 MiB chunks interleaved into the same
    rings ahead of the groups that consume them.
  - all elementwise/reduce work is batched per group (one op per group
    instead of one per block) and spread across vector/scalar/gpsimd so no
    engine comes near the DMA floor.

Layout: each core owns BC=8192 rows; block k (0..63), partition p holds local
row p*64+k, which makes every grouped load contiguous per partition and the
final [BC,2] output a single contiguous DMA from a [128, 128] staging tile.
"""

import numpy as np
from contextlib import ExitStack

import concourse.bacc as bacc
import concourse.bass as bass
import concourse.tile as tile
from concourse import mybir
from concourse.bass_utils import run_bass_kernel_spmd

GAMMA = 0.99
BETA = 0.01
B, T, A = 65536, 128, 8
N_CORES = 8
BC = B // N_CORES

G = 4  # row-blocks per streamed load group

F32 = mybir.dt.float32
ALU = mybir.AluOpType
ACTF = mybir.ActivationFunctionType
AXIS_X = mybir.AxisListType.X


def _discount_matrix() -> np.ndarray:
    # L[s, t] = gamma^(s-t) for s >= t else 0
    s = np.arange(T, dtype=np.float64)[:, None]
    t = np.arange(T, dtype=np.float64)[None, :]
    m = np.where(s >= t, GAMMA ** np.maximum(s - t, 0.0), 0.0)
    return m.astype(np.float32)


def build_nc(bc: int = BC):
    kb = bc // 128          # row-blocks
    ng = kb // G            # streamed groups
    nch = max(1, kb // 16)  # v/r chunks (16 blocks = 1 MiB of rewards each)
    jb = kb // nch          # blocks per chunk
    gpc = ng // nch         # groups per chunk
    assert bc % 128 == 0 and kb % G == 0 and ng % nch == 0 and jb % G == 0

    nc = bacc.Bacc("TRN2", target_bir_lowering=False, debug=False)

    v_d = nc.dram_tensor("values", [bc, T], F32, kind="ExternalInput")
    lv_d = nc.dram_tensor("last_value", [bc], F32, kind="ExternalInput")
    r_d = nc.dram_tensor("rewards", [bc, T], F32, kind="ExternalInput")
    lp_d = nc.dram_tensor("log_probs", [bc, T, A], F32, kind="ExternalInput")
    en_d = nc.dram_tensor("entropies", [bc, T, A], F32, kind="ExternalInput")
    tm_d = nc.dram_tensor("terminal_mask", [bc], mybir.dt.uint8, kind="ExternalInput")
    out_d = nc.dram_tensor("out", [bc, 2], F32, kind="ExternalOutput")

    lgam_d = nc.inline_tensor(_discount_matrix(), "lgam")
    iden_d = nc.inline_tensor(np.eye(128, dtype=np.float32), "iden")

    # local row index = p*kb + g*G + b -> grouped loads are contiguous per
    # partition (G and jb consecutive rows respectively)
    lp_view = lp_d.rearrange("(p g b) t a -> g p (b t) a", g=ng, b=G)
    en_view = en_d.rearrange("(p g b) t a -> g p b (t a)", g=ng, b=G)
    v_view = v_d.rearrange("(p c j) t -> c p (j t)", c=nch, j=jb)
    r_view = r_d.rearrange("(p c j) t -> c p j t", c=nch, j=jb)
    lv_view = lv_d.rearrange("(p k) -> p k", k=kb)
    tm_view = tm_d.rearrange("(p k) -> p k", k=kb)
    out_view = out_d.rearrange("(p k) j -> p (k j)", k=kb)

    with tile.TileContext(nc) as tc, ExitStack() as ctx:
        singles = ctx.enter_context(tc.tile_pool(name="singles", bufs=1))
        lp_pool = ctx.enter_context(tc.tile_pool(name="lp", bufs=3))
        en_pool = ctx.enter_context(tc.tile_pool(name="en", bufs=3))
        vr_pool = ctx.enter_context(tc.tile_pool(name="vr", bufs=nch))
        work = ctx.enter_context(tc.tile_pool(name="work", bufs=2))
        scr = ctx.enter_context(tc.tile_pool(name="scr", bufs=1))
        small = ctx.enter_context(tc.tile_pool(name="small", bufs=4))
        psum_t = ctx.enter_context(tc.tile_pool(name="psum_t", bufs=2, space="PSUM"))
        psum_r = ctx.enter_context(tc.tile_pool(name="psum_r", bufs=2, space="PSUM"))

        # singles go through SWDGE (gpsimd) so the two HWDGE rings are free
        # for the streamed loads from instruction zero
        lgam_s = singles.tile([128, 128], F32)
        nc.gpsimd.dma_start(out=lgam_s, in_=lgam_d[:])
        iden_s = singles.tile([128, 128], F32)
        nc.gpsimd.dma_start(out=iden_s, in_=iden_d[:])
        lv_s = singles.tile([128, kb], F32)
        nc.gpsimd.dma_start(out=lv_s, in_=lv_view)
        tm_s = singles.tile([128, kb], mybir.dt.uint8)
        nc.gpsimd.dma_start(out=tm_s, in_=tm_view)

        # gr0 = gamma * last_value * (1 - mask)
        tmf = singles.tile([128, kb], F32)
        nc.gpsimd.tensor_copy(out=tmf, in_=tm_s)
        lvm = singles.tile([128, kb], F32)
        nc.gpsimd.tensor_mul(lvm, lv_s, tmf)
        gr0 = singles.tile([128, kb], F32)
        nc.gpsimd.tensor_sub(gr0, lv_s, lvm)
        nc.gpsimd.tensor_scalar_mul(gr0, gr0, GAMMA)

        stage = singles.tile([128, 2 * kb], F32)
        stage_kj = stage.rearrange("p (k j) -> p k j", j=2)

        v_t: list = []
        r_t: list = []

        for g in range(ng):
            if g % gpc == 0:
                # prefetch the next v/r chunk into each ring ahead of the
                # groups that consume it; fold gamma*R0 into the last reward
                # column once per chunk (one strided op instead of per-block)
                c = g // gpc
                rt = vr_pool.tile([128, jb, T], F32)
                nc.sync.dma_start(out=rt, in_=r_view[c])
                vt = vr_pool.tile([128, jb * T], F32)
                nc.scalar.dma_start(out=vt, in_=v_view[c])
                nc.gpsimd.tensor_tensor(
                    out=rt[:, :, T - 1],
                    in0=rt[:, :, T - 1],
                    in1=gr0[:, c * jb : (c + 1) * jb],
                    op=ALU.add,
                )
                r_t.append(rt)
                v_t.append(vt)

            c = g // gpc
            j0 = (g % gpc) * G
            k0 = g * G

            lp_t = lp_pool.tile([128, G * T, A], F32)
            nc.sync.dma_start(out=lp_t, in_=lp_view[g])
            en_t = en_pool.tile([128, G, T * A], F32)
            nc.scalar.dma_start(out=en_t, in_=en_view[g])

            # lp[b, t] = sum_a log_probs
            lps = work.tile([128, G * T], F32)
            nc.vector.reduce_sum(out=lps, in_=lp_t, axis=AXIS_X)

            # nbe[b] = -beta * sum_{t,a} entropies on the scalar engine.
            # Scalar's whole per-group stream ([dma][4x ent][rt copy]) must
            # depend only on DMA and the tensor engine — never on vector —
            # or the ent/v ring stalls behind the vector loop. The scratch
            # out is a write-only dummy shared across blocks (scalar is
            # in-order so the WAW is free).
            ents = small.tile([128, G], F32)
            entscr = scr.tile([128, T * A], F32)
            for b in range(G):
                nc.scalar.activation(
                    out=entscr,
                    in_=en_t[:, b, :],
                    func=ACTF.Copy, bias=0.0, scale=-BETA,
                    accum_out=ents[:, b : b + 1],
                )

            # time-major rewards for the scan matmuls
            rt_ps = psum_t.tile([128, G * T], F32)
            for b in range(G):
                nc.tensor.transpose(
                    rt_ps[:, b * T : (b + 1) * T], r_t[c][:, j0 + b, :], iden_s
                )
            rt_sb = work.tile([128, G * T], F32)
            nc.scalar.activation(out=rt_sb, in_=rt_ps, func=ACTF.Copy, bias=0.0, scale=1.0)

            # R[b, t] = sum_s r'T[s, b] * Lgam[s, t]
            R_ps = psum_r.tile([128, G * T], F32)
            for b in range(G):
                nc.tensor.matmul(
                    R_ps[:, b * T : (b + 1) * T],
                    lhsT=rt_sb[:, b * T : (b + 1) * T],
                    rhs=lgam_s,
                    start=True,
                    stop=True,
                )

            adv = work.tile([128, G * T], F32)
            nc.vector.tensor_sub(adv, R_ps, v_t[c][:, j0 * T : (j0 + G) * T])

            # critic = 0.5 * sum_t adv^2 (square on vector, not scalar — see
            # the ents comment)
            sq = work.tile([128, G * T], F32)
            nc.vector.scalar_tensor_tensor(
                out=sq, in0=adv, scalar=0.5, in1=adv,
                op0=ALU.mult, op1=ALU.mult,
            )
            nc.vector.reduce_sum(
                out=stage_kj[:, k0 : k0 + G, 1],
                in_=sq.rearrange("p (b t) -> p b t", t=T),
                axis=AXIS_X,
            )

            # actor = -sum_t lp*adv - beta*sum ent
            prod = work.tile([128, G * T], F32)
            nc.vector.scalar_tensor_tensor(
                out=prod, in0=adv, scalar=-1.0, in1=lps,
                op0=ALU.mult, op1=ALU.mult,
            )
            acc = small.tile([128, G], F32)
            nc.vector.reduce_sum(
                out=acc, in_=prod.rearrange("p (b t) -> p b t", t=T), axis=AXIS_X
            )
            nc.vector.tensor_tensor(
                out=stage_kj[:, k0 : k0 + G, 0], in0=ents, in1=acc, op=ALU.add,
            )

        # single store at the end on the sync ring (drained of loads by then)
        nc.sync.dma_start(out=out_view, in_=stage)

    nc.compile()
    return nc


_NC = None


def _get_nc():
    global _NC
    if _NC is None:
        _NC = build_nc(BC)
    return _NC


def _make_in_maps(inputs: dict) -> list[dict]:
    v = np.ascontiguousarray(np.asarray(inputs["values"], dtype=np.float32))
    lv = np.ascontiguousarray(np.asarray(inputs["last_value"], dtype=np.float32))
    r = np.ascontiguousarray(np.asarray(inputs["rewards"], dtype=np.float32))
    lp = np.ascontiguousarray(np.asarray(inputs["log_probs"], dtype=np.float32))
    en = np.ascontiguousarray(np.asarray(inputs["entropies"], dtype=np.float32))
    tm = np.ascontiguousarray(np.asarray(inputs["terminal_mask"]).astype(np.uint8))
    maps = []
    for c in range(N_CORES):
        sl = slice(c * BC, (c + 1) * BC)
        maps.append(
            {
                "values": v[sl],
                "last_value": lv[sl],
                "rewards": r[sl],
                "log_probs": lp[sl],
                "entropies": en[sl],
                "terminal_mask": tm[sl],
            }
        )
    return maps


def _run(inputs: dict, trace: bool = False):
    nc = _get_nc()
    res = run_bass_kernel_spmd(
        nc,
        _make_in_maps(inputs),
        core_ids=list(range(N_CORES)),
        trace=trace,
    )
    out = np.concatenate([res.results[c]["out"] for c in range(N_CORES)], axis=0)
    return out, res


def kernel(**inputs) -> np.ndarray:
    out, _ = _run(inputs, trace=False)
    return out
